# revision 17
# baseline (speedup 1.0000x reference)
"""CrossAttention Trainium2 kernel.

Sharding: tensor-parallel over heads. Each of the 8 cores owns 2 of the 16
heads end-to-end: q/k/v projections for its 128 channels, SDPA for its heads
over the full sequence, and the out-projection contribution of its channels
(out_proj row-sharded); the 8 partial outputs are summed on the host.

Per-core device program (fp16 matmuls, fp32 PSUM accumulation):
  - qT/kvT arrive pre-transposed [hid, tok] so projection matmuls contract
    over the partition dim.
  - RMSNorm: squares on DVE (fp16, 4x mode), sum via a ones-vector matmul
    (partition-dim reduce on the PE), rsqrt via a linear seed + one Newton
    step (xs on DVE since it reads PSUM, the rest on the Pool engine).
    w_norm is folded into w_q on the host.
  - V is projected directly into natural [kv, ch] layout (input tile as the
    stationary operand, N=128 ch free) — no DMA transposes. A ones column is
    appended so row 64 of the P@V accumulator is the softmax denominator.
  - Scores are computed transposed ([kv, q]) so P^T feeds the P@V matmul
    directly; exp runs on ACT with the 1/sqrt(D) scale folded in. No
    max-subtraction: logits are O(6), well within fp16/fp32 exp range.
  - Optionally (OPTS['pv8']) P^T and V-ext are stored fp8e4 and P@V runs
    DoubleRow over kv-tile pairs.
  - out_proj emits outT [hid, tok] fp16 partials, one batched DMA per
    512-token window; host sums the 8 partials in fp32.
"""

from contextlib import ExitStack

import numpy as np
import ml_dtypes

import concourse.bacc as bacc
import concourse.bass as bass
import concourse.mybir as mybir
import concourse.tile as tile
from concourse import bass_utils

N_CORES = 8
B, SEQ, HID = 2, 2048, 1024
TOK = B * SEQ            # 4096
NH, D = 16, 64
CH = 128                 # q/k/v channels per core (2 heads)
HC = HID // 128          # 8 hidden chunks of 128
PT = 512                 # projection tile (tokens)
NPT = TOK // PT          # 8
KT = SEQ // 128          # 16 kv tiles of 128 per batch
GKT = B * KT             # 32 kv tiles globally
QW = 512                 # query window per scores tile
NQT = SEQ // QW          # 4
VW = 72                  # vext inner stride (64 ch + ones col + pad)
EPS = 1e-5
F16 = mybir.dt.float16
F32 = mybir.dt.float32
F8 = mybir.dt.float8e4
AF = mybir.ActivationFunctionType
ALU = mybir.AluOpType
DR = mybir.MatmulPerfMode.DoubleRow


OPTS = {"pv8": False, "a2a": True}
TA = 256                 # tokens per core per batch-half after the A2A


def emit_body(tc, t_aps, parts="abc"):
    nc = tc.nc
    qT = t_aps["qT"]
    kvT = t_aps["kvT"]
    outT = t_aps["outT"]
    pv8 = OPTS["pv8"]
    a2a = OPTS["a2a"] and parts == "abc"
    vdt = F8 if pv8 else F16

    with ExitStack() as ctx:
        singles = ctx.enter_context(tc.tile_pool(name="singles", bufs=1))
        qin = ctx.enter_context(tc.tile_pool(name="qin", bufs=4))
        sqp = ctx.enter_context(tc.tile_pool(name="sqp", bufs=2))
        small = ctx.enter_context(tc.tile_pool(name="small", bufs=3))
        rstdp = ctx.enter_context(tc.tile_pool(name="rstdp", bufs=2))
        pTp = ctx.enter_context(tc.tile_pool(name="pTp", bufs=3))
        denp = ctx.enter_context(tc.tile_pool(name="denp", bufs=2))
        obp = ctx.enter_context(tc.tile_pool(name="obp", bufs=2))
        pp = ctx.enter_context(tc.tile_pool(name="pp", bufs=2, space="PSUM"))
        sp = ctx.enter_context(tc.tile_pool(name="sp", bufs=2, space="PSUM"))
        op = ctx.enter_context(tc.tile_pool(name="op", bufs=2, space="PSUM"))

        # resident weights / activations
        wq_sb = singles.tile([128, HC, CH], F16, tag="wq")
        wk_sb = singles.tile([128, HC, CH], F16, tag="wk")
        wv_sb = singles.tile([128, HC, CH], F16, tag="wv")
        if a2a:
            # full out-proj weight: [in-ch part, in-chunk, out-chunk, out]
            wo_sb = singles.tile([128, HC, HC, 128], F16, tag="wo")
            ofp = ctx.enter_context(tc.tile_pool(name="ofp", bufs=2))
            ob2p = ctx.enter_context(tc.tile_pool(name="ob2p", bufs=2))
        else:
            wo_sb = singles.tile([CH, HC, 128], F16, tag="wo")
        bq_sb = singles.tile([128, 1], F32, tag="bq")
        bk_sb = singles.tile([128, 1], F32, tag="bk")
        bvrow = singles.tile([1, CH], F32, tag="bvrow")
        bvb_sb = singles.tile([128, 2, D], F32, tag="bvb")
        ones_sb = singles.tile([128, 1], F16, tag="ones")
        eps_sb = singles.tile([1, 1], F32, tag="eps")
        # fp8 P: exp offset keeps exp(logit - OFF) within e4m3 range; the
        # offset cancels exactly in the softmax normalization
        if pv8:
            off_sb = singles.tile([128, 1], F32, tag="off")
        else:
            off_sb = None
        kp_sb = singles.tile([128, TOK], F16, tag="kp")
        qp_sb = singles.tile([128, TOK], F16, tag="qp")
        # natural-layout V (+ ones col) per (kv-tile, head): [kv, g, h, ch]
        vext_sb = singles.tile([128, GKT, 2, VW], vdt, tag="vext")
        o_sb = singles.tile([128, TOK], F16, tag="osb")

        nc.sync.dma_start(wq_sb[:], t_aps["wqT"])
        nc.sync.dma_start(wk_sb[:], t_aps["wkT"])
        nc.sync.dma_start(wv_sb[:], t_aps["wvT"])
        nc.sync.dma_start(wo_sb[:], t_aps["woT"])
        nc.sync.dma_start(bq_sb[:], t_aps["bq"])
        nc.sync.dma_start(bk_sb[:], t_aps["bk"])
        nc.sync.dma_start(bvrow[:], t_aps["bv"])
        nc.gpsimd.partition_broadcast(bvb_sb[:], bvrow[:])
        nc.vector.memset(ones_sb[:], 1.0)
        nc.vector.memset(eps_sb[:], EPS)
        if pv8:
            nc.vector.memset(off_sb[:], -2.5)
        nc.vector.memset(vext_sb[:, :, :, D : D + 1], 1.0)

        # ---- Phase A: projections + RMSNorm stats, tiled over tokens ----
        def phase_a(t):
            ts = t * PT
            qt_t = qin.tile([128, HC, PT], F16, tag="qt")
            kvt_t = qin.tile([128, HC, PT], F16, tag="kvt")
            nc.sync.dma_start(qt_t[:], qT[:, :, ts : ts + PT])
            nc.sync.dma_start(kvt_t[:], kvT[:, :, ts : ts + PT])

            # sum of squares over hidden via ones-matmul (partition reduce);
            # squares on DVE (fp16 in/out, SBUF-only -> 4x mode)
            sq_t = sqp.tile([128, HC, PT], F16, tag="sq")
            nc.vector.tensor_mul(sq_t[:], qt_t[:], qt_t[:])
            ms_ps = pp.tile([1, PT], F32, tag="pp")
            for c in range(HC):
                nc.tensor.matmul(
                    ms_ps[:], ones_sb[:], sq_t[:, c, :],
                    start=(c == 0), stop=(c == HC - 1),
                )
            # rstd = 1/sqrt(ms/HID + eps) on DVE: ms/HID is within a few % of
            # 1.0, so a linear seed + one fused Newton step reaches ~5e-4.
            # (Kept off ACT so its activation table stays pinned to Exp.)
            xs = small.tile([1, PT], F32, tag="xs")
            nc.vector.tensor_scalar(
                xs[:], ms_ps[:], 1.0 / HID, EPS, ALU.mult, ALU.add
            )
            y0 = small.tile([1, PT], F32, tag="y0")
            nc.vector.tensor_scalar(y0[:], xs[:], -0.5, 1.5, ALU.mult, ALU.add)
            u = small.tile([1, PT], F32, tag="u")
            nc.vector.tensor_mul(u[:], y0[:], y0[:])
            nc.vector.scalar_tensor_tensor(
                u[:], u[:], -0.5, xs[:], op0=ALU.mult, op1=ALU.mult
            )
            y = small.tile([1, PT], F32, tag="y")
            nc.vector.scalar_tensor_tensor(
                y[:], u[:], 1.5, y0[:], op0=ALU.add, op1=ALU.mult
            )
            rstd_b = rstdp.tile([128, PT], F32, tag="rstd_b")
            nc.gpsimd.partition_broadcast(rstd_b[:], y[:])

            # k-projection -> K^T [ch, tok]
            kp_ps = pp.tile([128, PT], F32, tag="pp")
            for c in range(HC):
                nc.tensor.matmul(
                    kp_ps[:], wk_sb[:, c, :], kvt_t[:, c, :],
                    start=(c == 0), stop=(c == HC - 1),
                )
            nc.vector.tensor_scalar_add(kp_sb[:, ts : ts + PT], kp_ps[:], bk_sb[:])

            # q-projection -> Q^T [ch, tok], scaled by rstd then + b_q
            qp_ps = pp.tile([128, PT], F32, tag="pp")
            for c in range(HC):
                nc.tensor.matmul(
                    qp_ps[:], wq_sb[:, c, :], qt_t[:, c, :],
                    start=(c == 0), stop=(c == HC - 1),
                )
            nc.vector.tensor_mul(
                qp_sb[:, ts : ts + PT], qp_ps[:], rstd_b[:]
            )
            nc.vector.tensor_scalar_add(
                qp_sb[:, ts : ts + PT], qp_sb[:, ts : ts + PT], bq_sb[:]
            )

            # v-projection directly into natural [kv, ch] layout: the input
            # tile is the stationary operand, w_v streams (N=128 ch)
            for i in range(PT // 128):
                vp_ps = pp.tile([128, 2, D], F32, tag="pp", name=f"vp{i}")
                for c in range(HC):
                    nc.tensor.matmul(
                        vp_ps[:],
                        kvt_t[:, c, i * 128 : (i + 1) * 128],
                        wv_sb[:, c, :],
                        start=(c == 0), stop=(c == HC - 1),
                    )
                g = t * (PT // 128) + i
                nc.vector.tensor_add(
                    vext_sb[:, g, :, 0:D], vp_ps[:], bvb_sb[:]
                )

        # ---- Phase B: attention per (batch, q-window) ----
        def phase_b(b_idx, qt):
            qs = b_idx * SEQ + qt * QW
            o_ps = [
                op.tile([D + 1, QW], F32, tag="op", name=f"o_ps{h}")
                for h in range(2)
            ]
            if pv8:
                for k2 in range(KT // 2):
                    pT = pTp.tile([128, 2, 2, QW], F8, tag="pT")
                    for j in range(2):
                        kt = 2 * k2 + j
                        kv0 = b_idx * SEQ + kt * 128
                        s_ps = sp.tile([128, 2, QW], F32, tag="sp")
                        for h in range(2):
                            nc.tensor.matmul(
                                s_ps[:, h, :],
                                kp_sb[h * D : (h + 1) * D, kv0 : kv0 + 128],
                                qp_sb[h * D : (h + 1) * D, qs : qs + QW],
                                start=True, stop=True,
                            )
                        nc.scalar.activation(
                            pT[:, j, :, :], s_ps[:], AF.Exp,
                            bias=off_sb[:], scale=D ** -0.5,
                        )
                    g0 = b_idx * KT + 2 * k2
                    for h in range(2):
                        nc.tensor.matmul(
                            o_ps[h][:],
                            vext_sb[:, g0 : g0 + 2, h, 0 : D + 1],
                            pT[:, :, h, :],
                            start=(k2 == 0), stop=(k2 == KT // 2 - 1),
                            perf_mode=DR,
                        )
            else:
                for kt in range(KT):
                    kv0 = b_idx * SEQ + kt * 128
                    s_ps = sp.tile([128, 2, QW], F32, tag="sp")
                    for h in range(2):
                        nc.tensor.matmul(
                            s_ps[:, h, :],
                            kp_sb[h * D : (h + 1) * D, kv0 : kv0 + 128],
                            qp_sb[h * D : (h + 1) * D, qs : qs + QW],
                            start=True, stop=True,
                        )
                    pT = pTp.tile([128, 2, QW], F16, tag="pT")
                    nc.scalar.activation(pT[:], s_ps[:], AF.Exp, scale=D ** -0.5)
                    g = b_idx * KT + kt
                    for h in range(2):
                        nc.tensor.matmul(
                            o_ps[h][:],
                            vext_sb[:, g, h, 0 : D + 1],
                            pT[:, h, :],
                            start=(kt == 0), stop=(kt == KT - 1),
                        )
            for h in range(2):
                recip = small.tile([1, QW], F32, tag="recip")
                nc.vector.reciprocal(recip[:], o_ps[h][D : D + 1, :])
                den = denp.tile([D, QW], F32, tag="den")
                nc.gpsimd.partition_broadcast(den[:], recip[:])
                nc.vector.tensor_mul(
                    o_sb[h * D : (h + 1) * D, qs : qs + QW],
                    o_ps[h][0:D, :], den[:],
                )
            if a2a:
                # ship this window's o to its A2A send blocks
                for j2 in range(QW // TA):
                    j = (QW * qt) // TA + j2
                    nc.sync.dma_start(
                        t_aps["o_send"][b_idx, j],
                        o_sb[:, qs + j2 * TA : qs + (j2 + 1) * TA],
                    )

        # out-projection for a q-window (contract over our 128 ch);
        # all 8 chunks staged into one SBUF tile -> a single DMA
        def phase_c(b_idx, qt):
            qs = b_idx * SEQ + qt * QW
            ob = obp.tile([128, HC, QW], F16, tag="ob")
            for m in range(HC):
                out_ps = pp.tile([128, QW], F32, tag="pp")
                nc.tensor.matmul(
                    out_ps[:], wo_sb[:, m, :], o_sb[:, qs : qs + QW],
                    start=True, stop=True,
                )
                nc.vector.tensor_copy(ob[:, m, :], out_ps[:])
            nc.sync.dma_start(outT[:, :, qs : qs + QW], ob[:])

        # ablation stubs: fill tensors a disabled phase would produce
        if "a" not in parts:
            nc.vector.memset(kp_sb[:], 0.01)
            nc.vector.memset(qp_sb[:], 0.01)
            nc.vector.memset(vext_sb[:], 0.01)
        if "b" not in parts:
            nc.vector.memset(o_sb[:], 0.01)

        # interleave: batch-0 projections, then batch-0 attention woven with
        # batch-1 projections (keeps ACT fed while PE finishes projections)
        if parts == "abc":
            for t in range(NPT // 2):
                phase_a(t)
            for qt in range(NQT):
                phase_a(NPT // 2 + qt)
                phase_b(0, qt)
                phase_c(0, qt)
            for qt in range(NQT):
                phase_b(1, qt)
                phase_c(1, qt)
        else:
            if "a" in parts:
                for t in range(NPT):
                    phase_a(t)
            for b_idx in range(B):
                for qt in range(NQT):
                    if "b" in parts:
                        phase_b(b_idx, qt)
                    if "c" in parts:
                        phase_c(b_idx, qt)
            if "c" not in parts:
                # still need an output write so the NEFF has a full I/O set
                for b_idx in range(B):
                    for qt in range(NQT):
                        qs = b_idx * SEQ + qt * QW
                        ob = obp.tile([128, HC, QW], F16, tag="ob")
                        nc.vector.memset(ob[:], 0.0)
                        nc.sync.dma_start(outT[:, :, qs : qs + QW], ob[:])


def build_program(loop_n=None, parts="abc", body_reps=1):
    nc = bacc.Bacc("TRN2", target_bir_lowering=False, debug=False,
                   num_devices=N_CORES)
    specs = [
        ("qT", (128, HC, TOK), F16, "ExternalInput"),
        ("kvT", (128, HC, TOK), F16, "ExternalInput"),
        ("wqT", (128, HC, CH), F16, "ExternalInput"),
        ("wkT", (128, HC, CH), F16, "ExternalInput"),
        ("wvT", (128, HC, CH), F16, "ExternalInput"),
        ("woT", (CH, HC, 128), F16, "ExternalInput"),
        ("bq", (128, 1), F32, "ExternalInput"),
        ("bk", (128, 1), F32, "ExternalInput"),
        ("bv", (1, CH), F32, "ExternalInput"),
        ("outT", (128, HC, TOK), F16, "ExternalOutput"),
    ]
    t_aps = {}
    for name, shape, dt_, kind in specs:
        t_aps[name] = nc.dram_tensor(name, shape, dt_, kind=kind).ap()
    with tile.TileContext(nc) as tc:
        if loop_n is not None:
            hints = (
                mybir.EngineType.PE, mybir.EngineType.DVE,
                mybir.EngineType.Activation, mybir.EngineType.Pool,
                mybir.EngineType.SP,
            )
            with tc.For_i(0, loop_n, 1, hint_engines=hints):
                for _ in range(body_reps):
                    emit_body(tc, t_aps, parts=parts)
        else:
            emit_body(tc, t_aps, parts=parts)
    nc.compile()
    return nc


def prep_inputs(q, kv, w_norm, w_q, b_q, w_kv, b_kv, w_out, b_out):
    """Host-side shard prep: transpose/cast/slice the full inputs per core."""
    f16 = ml_dtypes.float16 if hasattr(ml_dtypes, "float16") else np.float16

    def to_chunked_T(x2d):
        # [tok, hid] -> [128, hid//128, tok]
        tok, hid = x2d.shape
        return np.ascontiguousarray(
            x2d.T.reshape(hid // 128, 128, tok).transpose(1, 0, 2)
        )

    q = np.asarray(q, np.float32)
    kv = np.asarray(kv, np.float32)
    w_norm = np.asarray(w_norm, np.float32)
    w_q = np.asarray(w_q, np.float32)
    b_q = np.asarray(b_q, np.float32)
    w_kv = np.asarray(w_kv, np.float32)
    b_kv = np.asarray(b_kv, np.float32)
    w_out = np.asarray(w_out, np.float32)

    qT = to_chunked_T(q.reshape(TOK, HID)).astype(f16)
    kvT = to_chunked_T(kv.reshape(TOK, HID)).astype(f16)
    w_q_eff = w_q * w_norm[None, :]

    in_maps = []
    for c in range(N_CORES):
        r0 = CH * c
        # [out_ch, in_hid] slices -> [128, HC, out_ch] chunked on in_hid
        wq_c = to_chunked_T(w_q_eff[r0 : r0 + CH]).astype(f16)
        wk_c = to_chunked_T(w_kv[r0 : r0 + CH]).astype(f16)
        wv_c = to_chunked_T(w_kv[HID + r0 : HID + r0 + CH]).astype(f16)
        wo_c = np.ascontiguousarray(
            w_out[:, r0 : r0 + CH].T.reshape(CH, HC, 128)
        ).astype(f16)
        in_maps.append({
            "qT": qT,
            "kvT": kvT,
            "wqT": wq_c,
            "wkT": wk_c,
            "wvT": wv_c,
            "woT": wo_c,
            "bq": b_q[r0 : r0 + CH].reshape(128, 1).copy(),
            "bk": b_kv[r0 : r0 + CH].reshape(128, 1).copy(),
            "bv": b_kv[HID + r0 : HID + r0 + CH].reshape(1, CH).copy(),
        })
    return in_maps


_CACHE = {}


def _get_nc():
    if "nc" not in _CACHE:
        _CACHE["nc"] = build_program()
    return _CACHE["nc"]


def gather_output(results, b_out):
    acc = np.zeros((HID, TOK), np.float64)
    for c in range(len(results)):
        # outT [128, HC, TOK]: out-channel = m*128 + p
        part = results[c]["outT"].astype(np.float64)
        acc += part.transpose(1, 0, 2).reshape(HID, TOK)
    out = acc.T.astype(np.float32) + np.asarray(b_out, np.float32)[None, :]
    return np.ascontiguousarray(out.reshape(B, SEQ, HID))


def kernel(q, kv, w_norm, w_q, b_q, w_kv, b_kv, w_out, b_out):
    nc = _get_nc()
    in_maps = prep_inputs(q, kv, w_norm, w_q, b_q, w_kv, b_kv, w_out, b_out)
    res = bass_utils.run_bass_kernel_spmd(nc, in_maps, core_ids=list(range(N_CORES)))
    return gather_output(res.results, b_out)


# revision 30
# speedup vs baseline: 1.4598x; 1.4598x over previous
"""CrossAttention Trainium2 kernel.

Sharding: tensor-parallel over heads. Each of the 8 cores owns 2 of the 16
heads end-to-end: q/k/v projections for its 128 channels, SDPA for its heads
over the full sequence, and the out-projection contribution of its channels
(out_proj row-sharded); the 8 partial outputs are summed on the host.

Per-core device program (fp16 matmuls, fp32 PSUM accumulation):
  - qT/kvT arrive pre-transposed [hid, tok] so projection matmuls contract
    over the partition dim.
  - RMSNorm: squares on DVE (fp16, 4x mode), sum via a ones-vector matmul
    (partition-dim reduce on the PE), rsqrt via a linear seed + one Newton
    step (xs on DVE since it reads PSUM, the rest on the Pool engine).
    w_norm is folded into w_q on the host.
  - V is projected directly into natural [kv, ch] layout (input tile as the
    stationary operand, N=128 ch free) — no DMA transposes. A ones column is
    appended so row 64 of the P@V accumulator is the softmax denominator.
  - Scores are computed transposed ([kv, q]) so P^T feeds the P@V matmul
    directly; exp runs on ACT with the 1/sqrt(D) scale folded in. No
    max-subtraction: logits are O(6), well within fp16/fp32 exp range.
  - Optionally (OPTS['pv8']) P^T and V-ext are stored fp8e4 and P@V runs
    DoubleRow over kv-tile pairs.
  - out_proj emits outT [hid, tok] fp16 partials, one batched DMA per
    512-token window; host sums the 8 partials in fp32.
"""

from contextlib import ExitStack

import numpy as np
import ml_dtypes

import concourse.bacc as bacc
import concourse.bass as bass
import concourse.mybir as mybir
import concourse.tile as tile
from concourse import bass_utils

N_CORES = 8
B, SEQ, HID = 2, 2048, 1024
TOK = B * SEQ            # 4096
NH, D = 16, 64
CH = 128                 # q/k/v channels per core (2 heads)
HC = HID // 128          # 8 hidden chunks of 128
PT = 512                 # projection tile (tokens)
NPT = TOK // PT          # 8
KT = SEQ // 128          # 16 kv tiles of 128 per batch
GKT = B * KT             # 32 kv tiles globally
QW = 512                 # query window per scores tile
NQT = SEQ // QW          # 4
VW = 72                  # vext inner stride (64 ch + ones col + pad)
EPS = 1e-5
F16 = mybir.dt.float16
F32 = mybir.dt.float32
F8 = mybir.dt.float8e4
AF = mybir.ActivationFunctionType
ALU = mybir.AluOpType
DR = mybir.MatmulPerfMode.DoubleRow


OPTS = {"pv8": False, "a2a": False}
TA = 256                 # tokens per core per batch-half after the A2A


def emit_body(tc, t_aps, parts="abc"):
    nc = tc.nc
    qT = t_aps["qT"]
    kvT = t_aps["kvT"]
    outT = t_aps["outT"]
    pv8 = OPTS["pv8"]
    a2a = OPTS["a2a"] and parts == "abc"
    vdt = F8 if pv8 else F16

    with ExitStack() as ctx:
        singles = ctx.enter_context(tc.tile_pool(name="singles", bufs=1))
        qin = ctx.enter_context(tc.tile_pool(name="qin", bufs=4))
        sqp = ctx.enter_context(tc.tile_pool(name="sqp", bufs=2))
        small = ctx.enter_context(tc.tile_pool(name="small", bufs=3))
        rstdp = ctx.enter_context(tc.tile_pool(name="rstdp", bufs=2))
        pTp = ctx.enter_context(tc.tile_pool(name="pTp", bufs=3))
        denp = ctx.enter_context(tc.tile_pool(name="denp", bufs=2))
        obp = ctx.enter_context(tc.tile_pool(name="obp", bufs=2))
        pp = ctx.enter_context(tc.tile_pool(name="pp", bufs=2, space="PSUM"))
        sp = ctx.enter_context(tc.tile_pool(name="sp", bufs=2, space="PSUM"))
        op = ctx.enter_context(tc.tile_pool(name="op", bufs=2, space="PSUM"))

        # resident weights / activations
        wq_sb = singles.tile([128, HC, CH], F16, tag="wq")
        wk_sb = singles.tile([128, HC, CH], F16, tag="wk")
        wv_sb = singles.tile([128, HC, CH], F16, tag="wv")
        if a2a:
            # full out-proj weight: [in-ch part, in-chunk, out-chunk, out]
            wo_sb = singles.tile([128, HC, HC, 128], F16, tag="wo")
            of2_sb = singles.tile([128, 2, HC, TA], F16, tag="of2")
            ob2p = ctx.enter_context(tc.tile_pool(name="ob2p", bufs=2))
        else:
            wo_sb = singles.tile([CH, HC, 128], F16, tag="wo")
        bq_sb = singles.tile([128, 1], F32, tag="bq")
        bk_sb = singles.tile([128, 1], F32, tag="bk")
        bvrow = singles.tile([1, CH], F32, tag="bvrow")
        bvb_sb = singles.tile([128, 2, D], F32, tag="bvb")
        ones_sb = singles.tile([128, 1], F16, tag="ones")
        eps_sb = singles.tile([1, 1], F32, tag="eps")
        # fp8 P: exp offset keeps exp(logit - OFF) within e4m3 range; the
        # offset cancels exactly in the softmax normalization
        if pv8:
            off_sb = singles.tile([128, 1], F32, tag="off")
        else:
            off_sb = None
        kp_sb = singles.tile([128, TOK], F16, tag="kp")
        qp_sb = singles.tile([128, TOK], F16, tag="qp")
        # natural-layout V (+ ones col) per (kv-tile, head): [kv, g, h, ch]
        vext_sb = singles.tile([128, GKT, 2, VW], vdt, tag="vext")
        o_sb = singles.tile([128, TOK], F16, tag="osb")

        nc.sync.dma_start(wq_sb[:], t_aps["wqT"])
        nc.sync.dma_start(wk_sb[:], t_aps["wkT"])
        nc.sync.dma_start(wv_sb[:], t_aps["wvT"])
        nc.sync.dma_start(wo_sb[:], t_aps["woT"])
        nc.sync.dma_start(bq_sb[:], t_aps["bq"])
        nc.sync.dma_start(bk_sb[:], t_aps["bk"])
        nc.sync.dma_start(bvrow[:], t_aps["bv"])
        nc.gpsimd.partition_broadcast(bvb_sb[:], bvrow[:])
        nc.vector.memset(ones_sb[:], 1.0)
        nc.vector.memset(eps_sb[:], EPS)
        if pv8:
            nc.vector.memset(off_sb[:], -2.5)
        nc.vector.memset(vext_sb[:, :, :, D : D + 1], 1.0)

        # ---- Phase A: projections + RMSNorm stats, tiled over tokens ----
        def phase_a(t):
            ts = t * PT
            qt_t = qin.tile([128, HC, PT], F16, tag="qt")
            kvt_t = qin.tile([128, HC, PT], F16, tag="kvt")
            nc.sync.dma_start(qt_t[:], qT[:, :, ts : ts + PT])
            nc.sync.dma_start(kvt_t[:], kvT[:, :, ts : ts + PT])

            # sum of squares over hidden via ones-matmul (partition reduce);
            # squares on DVE (fp16 in/out, SBUF-only -> 4x mode)
            sq_t = sqp.tile([128, HC, PT], F16, tag="sq")
            nc.vector.tensor_mul(sq_t[:], qt_t[:], qt_t[:])
            ms_ps = pp.tile([1, PT], F32, tag="pp")
            for c in range(HC):
                nc.tensor.matmul(
                    ms_ps[:], ones_sb[:], sq_t[:, c, :],
                    start=(c == 0), stop=(c == HC - 1),
                )
            # rstd = 1/sqrt(ms/HID + eps): sqrt on ACT (reads PSUM, folds
            # scale+eps), reciprocal on DVE. Measured faster than a DVE
            # Newton chain (341us vs 390us) — ACT keeps both tables live.
            sx = small.tile([1, PT], F32, tag="sx")
            nc.scalar.activation(
                sx[:], ms_ps[:], AF.Sqrt, bias=eps_sb[:], scale=1.0 / HID
            )
            y = small.tile([1, PT], F32, tag="y")
            nc.vector.reciprocal(y[:], sx[:])
            rstd_b = rstdp.tile([128, PT], F32, tag="rstd_b")
            nc.gpsimd.partition_broadcast(rstd_b[:], y[:])

            # k-projection -> K^T [ch, tok]
            kp_ps = pp.tile([128, PT], F32, tag="pp")
            for c in range(HC):
                nc.tensor.matmul(
                    kp_ps[:], wk_sb[:, c, :], kvt_t[:, c, :],
                    start=(c == 0), stop=(c == HC - 1),
                )
            nc.vector.tensor_scalar_add(kp_sb[:, ts : ts + PT], kp_ps[:], bk_sb[:])

            # q-projection -> Q^T [ch, tok], scaled by rstd then + b_q
            qp_ps = pp.tile([128, PT], F32, tag="pp")
            for c in range(HC):
                nc.tensor.matmul(
                    qp_ps[:], wq_sb[:, c, :], qt_t[:, c, :],
                    start=(c == 0), stop=(c == HC - 1),
                )
            nc.vector.tensor_mul(
                qp_sb[:, ts : ts + PT], qp_ps[:], rstd_b[:]
            )
            nc.vector.tensor_scalar_add(
                qp_sb[:, ts : ts + PT], qp_sb[:, ts : ts + PT], bq_sb[:]
            )

            # v-projection directly into natural [kv, ch] layout: the input
            # tile is the stationary operand, w_v streams (N=128 ch)
            for i in range(PT // 128):
                vp_ps = pp.tile([128, 2, D], F32, tag="pp", name=f"vp{i}")
                for c in range(HC):
                    nc.tensor.matmul(
                        vp_ps[:],
                        kvt_t[:, c, i * 128 : (i + 1) * 128],
                        wv_sb[:, c, :],
                        start=(c == 0), stop=(c == HC - 1),
                    )
                g = t * (PT // 128) + i
                nc.vector.tensor_add(
                    vext_sb[:, g, :, 0:D], vp_ps[:], bvb_sb[:]
                )

        # ---- Phase B: attention per (batch, q-window) ----
        o_ps_live = {}

        def phase_b_chunk(b_idx, qt, kt_lo, kt_hi):
            qs = b_idx * SEQ + qt * QW
            if kt_lo == 0:
                o_ps_live[(b_idx, qt)] = [
                    op.tile([D + 1, QW], F32, tag="op",
                            name=f"o_ps{b_idx}_{qt}_{h}")
                    for h in range(2)
                ]
            o_ps = o_ps_live[(b_idx, qt)]
            for kt in range(kt_lo, kt_hi):
                kv0 = b_idx * SEQ + kt * 128
                s_ps = sp.tile([128, 2, QW], F32, tag="sp")
                for h in range(2):
                    nc.tensor.matmul(
                        s_ps[:, h, :],
                        kp_sb[h * D : (h + 1) * D, kv0 : kv0 + 128],
                        qp_sb[h * D : (h + 1) * D, qs : qs + QW],
                        start=True, stop=True,
                    )
                pT = pTp.tile([128, 2, QW], F16, tag="pT")
                nc.scalar.activation(pT[:], s_ps[:], AF.Exp, scale=D ** -0.5)
                g = b_idx * KT + kt
                for h in range(2):
                    nc.tensor.matmul(
                        o_ps[h][:],
                        vext_sb[:, g, h, 0 : D + 1],
                        pT[:, h, :],
                        start=(kt == 0), stop=(kt == KT - 1),
                    )
            if kt_hi == KT:
                for h in range(2):
                    recip = small.tile([1, QW], F32, tag="recip")
                    nc.vector.reciprocal(recip[:], o_ps[h][D : D + 1, :])
                    den = denp.tile([D, QW], F32, tag="den")
                    nc.gpsimd.partition_broadcast(den[:], recip[:])
                    nc.vector.tensor_mul(
                        o_sb[h * D : (h + 1) * D, qs : qs + QW],
                        o_ps[h][0:D, :], den[:],
                    )
                del o_ps_live[(b_idx, qt)]
                if a2a:
                    for j2 in range(QW // TA):
                        j = (QW * qt) // TA + j2
                        nc.sync.dma_start(
                            t_aps["o_send"][b_idx, j],
                            o_sb[:, qs + j2 * TA : qs + (j2 + 1) * TA],
                        )

        def phase_b(b_idx, qt):
            if not pv8:
                phase_b_chunk(b_idx, qt, 0, KT)
                return
            qs = b_idx * SEQ + qt * QW
            o_ps = [
                op.tile([D + 1, QW], F32, tag="op", name=f"o_ps{h}")
                for h in range(2)
            ]
            if pv8:
                for k2 in range(KT // 2):
                    pT = pTp.tile([128, 2, 2, QW], F8, tag="pT")
                    for j in range(2):
                        kt = 2 * k2 + j
                        kv0 = b_idx * SEQ + kt * 128
                        s_ps = sp.tile([128, 2, QW], F32, tag="sp")
                        for h in range(2):
                            nc.tensor.matmul(
                                s_ps[:, h, :],
                                kp_sb[h * D : (h + 1) * D, kv0 : kv0 + 128],
                                qp_sb[h * D : (h + 1) * D, qs : qs + QW],
                                start=True, stop=True,
                            )
                        nc.scalar.activation(
                            pT[:, j, :, :], s_ps[:], AF.Exp,
                            bias=off_sb[:], scale=D ** -0.5,
                        )
                    g0 = b_idx * KT + 2 * k2
                    for h in range(2):
                        nc.tensor.matmul(
                            o_ps[h][:],
                            vext_sb[:, g0 : g0 + 2, h, 0 : D + 1],
                            pT[:, :, h, :],
                            start=(k2 == 0), stop=(k2 == KT // 2 - 1),
                            perf_mode=DR,
                        )
            else:
                for kt in range(KT):
                    kv0 = b_idx * SEQ + kt * 128
                    s_ps = sp.tile([128, 2, QW], F32, tag="sp")
                    for h in range(2):
                        nc.tensor.matmul(
                            s_ps[:, h, :],
                            kp_sb[h * D : (h + 1) * D, kv0 : kv0 + 128],
                            qp_sb[h * D : (h + 1) * D, qs : qs + QW],
                            start=True, stop=True,
                        )
                    pT = pTp.tile([128, 2, QW], F16, tag="pT")
                    nc.scalar.activation(pT[:], s_ps[:], AF.Exp, scale=D ** -0.5)
                    g = b_idx * KT + kt
                    for h in range(2):
                        nc.tensor.matmul(
                            o_ps[h][:],
                            vext_sb[:, g, h, 0 : D + 1],
                            pT[:, h, :],
                            start=(kt == 0), stop=(kt == KT - 1),
                        )
            for h in range(2):
                recip = small.tile([1, QW], F32, tag="recip")
                nc.vector.reciprocal(recip[:], o_ps[h][D : D + 1, :])
                den = denp.tile([D, QW], F32, tag="den")
                nc.gpsimd.partition_broadcast(den[:], recip[:])
                nc.vector.tensor_mul(
                    o_sb[h * D : (h + 1) * D, qs : qs + QW],
                    o_ps[h][0:D, :], den[:],
                )
            if a2a:
                # ship this window's o to its A2A send blocks
                for j2 in range(QW // TA):
                    j = (QW * qt) // TA + j2
                    nc.sync.dma_start(
                        t_aps["o_send"][b_idx, j],
                        o_sb[:, qs + j2 * TA : qs + (j2 + 1) * TA],
                    )

        # out-projection for a q-window (contract over our 128 ch);
        # all 8 chunks staged into one SBUF tile -> a single DMA
        def phase_c(b_idx, qt):
            qs = b_idx * SEQ + qt * QW
            ob = obp.tile([128, HC, QW], F16, tag="ob")
            for m in range(HC):
                out_ps = pp.tile([128, QW], F32, tag="pp")
                nc.tensor.matmul(
                    out_ps[:], wo_sb[:, m, :], o_sb[:, qs : qs + QW],
                    start=True, stop=True,
                )
                nc.vector.tensor_copy(ob[:, m, :], out_ps[:])
            nc.sync.dma_start(outT[:, :, qs : qs + QW], ob[:])

        # A2A path: exchange o so this core holds all 1024 channels for its
        # TA-token slice of batch-half bh, then out-project with full K
        def a2a_exchange(bh):
            nc.gpsimd.collective_compute(
                "AllToAll",
                mybir.AluOpType.bypass,
                replica_groups=[list(range(N_CORES))],
                ins=[t_aps["o_send"][bh]],
                outs=[t_aps["o_recv"][bh]],
            )

        def a2a_recv(bh):
            for k in range(HC):
                nc.sync.dma_start(of2_sb[:, bh, k, :], t_aps["o_recv"][bh, k])

        def phase_c2(bh, ms):
            # out-proj chunks ms (list of m indices) for our token slice
            ob = ob2p.tile([128, len(ms), TA], F16, tag="ob2",
                           name=f"ob2_{bh}_{ms[0]}")
            for mi, m in enumerate(ms):
                out_ps = pp.tile([128, TA], F32, tag="pp", name=f"o2_{bh}_{m}")
                for k in range(HC):
                    nc.tensor.matmul(
                        out_ps[:], wo_sb[:, k, m, :], of2_sb[:, bh, k, :],
                        start=(k == 0), stop=(k == HC - 1),
                    )
                nc.vector.tensor_copy(ob[:, mi, :], out_ps[:])
            nc.sync.dma_start(outT[:, ms[0] : ms[0] + len(ms), bh, :], ob[:])

        # ablation stubs: fill tensors a disabled phase would produce
        if "a" not in parts:
            nc.vector.memset(kp_sb[:], 0.01)
            nc.vector.memset(qp_sb[:], 0.01)
            nc.vector.memset(vext_sb[:], 0.01)
        if "b" not in parts:
            nc.vector.memset(o_sb[:], 0.01)

        # interleave: batch-0 projections, then batch-0 attention woven with
        # batch-1 projections (keeps ACT fed while PE finishes projections)
        if parts == "abc" and a2a:
            for t in range(NPT // 2):
                phase_a(t)
            for qt in range(NQT):
                phase_a(NPT // 2 + qt)
                phase_b(0, qt)
            a2a_exchange(0)
            a2a_recv(0)
            # batch-0 out-proj (post-A2A) woven into batch-1 attention
            for qt in range(NQT):
                phase_b(1, qt)
                phase_c2(0, [2 * qt, 2 * qt + 1])
            a2a_exchange(1)
            a2a_recv(1)
            for mg in range(NQT):
                phase_c2(1, [2 * mg, 2 * mg + 1])
        elif parts == "abc" and not pv8:
            # kt-granular head: window (0,0) starts after 2 projection tiles
            phase_a(0)
            phase_a(1)
            phase_b_chunk(0, 0, 0, 8)
            phase_a(2)
            phase_b_chunk(0, 0, 8, 12)
            phase_a(3)
            phase_b_chunk(0, 0, 12, KT)
            phase_a(4)
            phase_b(0, 1)
            phase_c(0, 0)
            phase_a(5)
            phase_b(0, 2)
            phase_c(0, 1)
            phase_a(6)
            phase_b(0, 3)
            phase_c(0, 2)
            phase_a(7)
            phase_b(1, 0)
            phase_c(0, 3)
            for qt in range(1, NQT):
                phase_b(1, qt)
                phase_c(1, qt - 1)
            phase_c(1, NQT - 1)
        elif parts == "abc":
            for t in range(NPT // 2):
                phase_a(t)
            for qt in range(NQT):
                phase_a(NPT // 2 + qt)
                phase_b(0, qt)
                phase_c(0, qt)
            for qt in range(NQT):
                phase_b(1, qt)
                phase_c(1, qt)
        else:
            if "a" in parts:
                for t in range(NPT):
                    phase_a(t)
            for b_idx in range(B):
                for qt in range(NQT):
                    if "b" in parts:
                        phase_b(b_idx, qt)
                    if "c" in parts:
                        phase_c(b_idx, qt)
            if "c" not in parts:
                # still need an output write so the NEFF has a full I/O set
                for b_idx in range(B):
                    for qt in range(NQT):
                        qs = b_idx * SEQ + qt * QW
                        ob = obp.tile([128, HC, QW], F16, tag="ob")
                        nc.vector.memset(ob[:], 0.0)
                        nc.sync.dma_start(outT[:, :, qs : qs + QW], ob[:])


def build_program(loop_n=None, parts="abc", body_reps=1):
    a2a = OPTS["a2a"] and parts == "abc"
    nc = bacc.Bacc("TRN2", target_bir_lowering=False, debug=False,
                   num_devices=N_CORES)
    specs = [
        ("qT", (128, HC, TOK), F16, "ExternalInput"),
        ("kvT", (128, HC, TOK), F16, "ExternalInput"),
        ("wqT", (128, HC, CH), F16, "ExternalInput"),
        ("wkT", (128, HC, CH), F16, "ExternalInput"),
        ("wvT", (128, HC, CH), F16, "ExternalInput"),
        ("bq", (128, 1), F32, "ExternalInput"),
        ("bk", (128, 1), F32, "ExternalInput"),
        ("bv", (1, CH), F32, "ExternalInput"),
    ]
    if a2a:
        specs += [
            ("woT", (128, HC, HC, 128), F16, "ExternalInput"),
            ("outT", (128, HC, 2, TA), F16, "ExternalOutput"),
        ]
    else:
        specs += [
            ("woT", (CH, HC, 128), F16, "ExternalInput"),
            ("outT", (128, HC, TOK), F16, "ExternalOutput"),
        ]
    t_aps = {}
    for name, shape, dt_, kind in specs:
        t_aps[name] = nc.dram_tensor(name, shape, dt_, kind=kind).ap()
    if a2a:
        for name in ("o_send", "o_recv"):
            t_aps[name] = nc.dram_tensor(
                name, (2, N_CORES, 128, TA), F16, kind="Internal"
            ).ap()
    with tile.TileContext(nc) as tc:
        if loop_n is not None:
            hints = (
                mybir.EngineType.PE, mybir.EngineType.DVE,
                mybir.EngineType.Activation, mybir.EngineType.Pool,
                mybir.EngineType.SP,
            )
            with tc.For_i(0, loop_n, 1, hint_engines=hints):
                for _ in range(body_reps):
                    emit_body(tc, t_aps, parts=parts)
        else:
            emit_body(tc, t_aps, parts=parts)
    nc.compile()
    return nc


def prep_inputs(q, kv, w_norm, w_q, b_q, w_kv, b_kv, w_out, b_out):
    """Host-side shard prep: transpose/cast/slice the full inputs per core."""
    f16 = ml_dtypes.float16 if hasattr(ml_dtypes, "float16") else np.float16

    def to_chunked_T(x2d):
        # [tok, hid] -> [128, hid//128, tok]
        tok, hid = x2d.shape
        return np.ascontiguousarray(
            x2d.T.reshape(hid // 128, 128, tok).transpose(1, 0, 2)
        )

    q = np.asarray(q, np.float32)
    kv = np.asarray(kv, np.float32)
    w_norm = np.asarray(w_norm, np.float32)
    w_q = np.asarray(w_q, np.float32)
    b_q = np.asarray(b_q, np.float32)
    w_kv = np.asarray(w_kv, np.float32)
    b_kv = np.asarray(b_kv, np.float32)
    w_out = np.asarray(w_out, np.float32)

    qT = to_chunked_T(q.reshape(TOK, HID)).astype(f16)
    kvT = to_chunked_T(kv.reshape(TOK, HID)).astype(f16)
    w_q_eff = w_q * w_norm[None, :]

    if OPTS["a2a"]:
        # full out-proj weight: [in-ch part p, in-chunk k, out-chunk m, out j]
        # woT[p, k, m, j] = w_out[m*128+j, k*128+p]
        wo_full = np.ascontiguousarray(
            w_out.reshape(HC, 128, HC, 128).transpose(3, 2, 0, 1)
        ).astype(f16)
        # wo_full[p, k, m, j] = w_out[m, j, k, p] after reshape -> transpose
    in_maps = []
    for c in range(N_CORES):
        r0 = CH * c
        # [out_ch, in_hid] slices -> [128, HC, out_ch] chunked on in_hid
        wq_c = to_chunked_T(w_q_eff[r0 : r0 + CH]).astype(f16)
        wk_c = to_chunked_T(w_kv[r0 : r0 + CH]).astype(f16)
        wv_c = to_chunked_T(w_kv[HID + r0 : HID + r0 + CH]).astype(f16)
        if OPTS["a2a"]:
            wo_c = wo_full
        else:
            wo_c = np.ascontiguousarray(
                w_out[:, r0 : r0 + CH].T.reshape(CH, HC, 128)
            ).astype(f16)
        in_maps.append({
            "qT": qT,
            "kvT": kvT,
            "wqT": wq_c,
            "wkT": wk_c,
            "wvT": wv_c,
            "woT": wo_c,
            "bq": b_q[r0 : r0 + CH].reshape(128, 1).copy(),
            "bk": b_kv[r0 : r0 + CH].reshape(128, 1).copy(),
            "bv": b_kv[HID + r0 : HID + r0 + CH].reshape(1, CH).copy(),
        })
    return in_maps


_CACHE = {}


def _get_nc():
    if "nc" not in _CACHE:
        _CACHE["nc"] = build_program()
    return _CACHE["nc"]


def gather_output(results, b_out):
    if OPTS["a2a"]:
        # each core owns TA tokens per batch-half: concat, no summation
        full = np.zeros((HID, TOK), np.float32)
        for c in range(len(results)):
            arr = np.asarray(results[c]["outT"], np.float32)  # [128,HC,2,TA]
            for bh in range(B):
                chunk = arr[:, :, bh, :].transpose(1, 0, 2).reshape(HID, TA)
                s = bh * SEQ + c * TA
                full[:, s : s + TA] = chunk
        out = full.T + np.asarray(b_out, np.float32)[None, :]
        return np.ascontiguousarray(out.reshape(B, SEQ, HID))
    acc = np.zeros((HID, TOK), np.float64)
    for c in range(len(results)):
        # outT [128, HC, TOK]: out-channel = m*128 + p
        part = results[c]["outT"].astype(np.float64)
        acc += part.transpose(1, 0, 2).reshape(HID, TOK)
    out = acc.T.astype(np.float32) + np.asarray(b_out, np.float32)[None, :]
    return np.ascontiguousarray(out.reshape(B, SEQ, HID))


def kernel(q, kv, w_norm, w_q, b_q, w_kv, b_kv, w_out, b_out):
    nc = _get_nc()
    in_maps = prep_inputs(q, kv, w_norm, w_q, b_q, w_kv, b_kv, w_out, b_out)
    res = bass_utils.run_bass_kernel_spmd(nc, in_maps, core_ids=list(range(N_CORES)))
    return gather_output(res.results, b_out)
